# revision 71
# baseline (speedup 1.0000x reference)
"""Trainium2 Bass kernel for ConvBnSign (binarized 3x3 conv + sync-BN + sign).

Math: y = conv2d(x, sign(w) * alpha)  with alpha = mean|w| per out-channel,
then train-mode BatchNorm over (N,H,W), then hard_sign.

Since alpha_o > 0 is a per-channel scale, fold it into the BN affine:
  z = conv2d(x, sign(w))
  out = sign(z*A + B),  A = alpha*gamma*rsqrt(alpha^2 var_z + eps),
                        B = beta - mu_z*A

Precision: x is split on host into 3 fp8-e4m3 planes (lev1 = e4m3(x),
lev2 = e4m3(r1*2^4), lev3 = e4m3(r2*2^8)); reconstruction error rms
~2^-16 relative, which flips only ~4e-6 of the output signs.

Conv engine schedule: every 128-contraction k-tile (9 taps x 3 levels = 27
per output tile, plus one zero-padded slot) is executed via fp8 DoubleRow
matmuls that process TWO k-tiles per instruction at 0.5 cycles/row:
  - 9 pairs  (lev1, lev2) of the same tap, pair-stride = PADPIX
  - 4 pairs  (lev3 tap 2i, lev3' tap 2i+1) using a duplicated lev3 plane
    so the pair stride stays collision-free
  - 1 pair   (lev3 tap 8, zero weights)
=> 14 matmuls per [128,448] PSUM tile instead of 18 bf16 matmuls.

Sharding: data-parallel, 4 images per core across 8 cores; BN stats are
per-channel partial sums [128,2] fp32 all-reduced across cores.

Other schedule details:
  - ~3us of tiny warmup matmuls on a zeroed tile bring the PE out of its
    low p-states while the first input DMAs are still in flight
  - input DMA order is tuned around the dep-tracker's contiguous-range
    granularity (img0 lev1 whole, then lev2 head rows, then the rest)
  - BN stats: z-copy+sum-accum on DVE from PSUM; square+sumsq-accum
    alternates ACT (from PSUM) / DVE (from the z copy; GPSIMD cannot
    access PSUM), with the last tile on DVE right behind its own copy
  - sign outputs are fp8 half/quarter-image slices spread over ACT
    (Sign -> ±1), DVE and Pool (is_ge(T) -> {0,1}, host decodes 2v-1),
    with out-DMAs issued through both HWDGE and Pool's SWDGE queues
"""

import numpy as np
import ml_dtypes
import bass_rust

import concourse.bass as bass
import concourse.mybir as mybir
import concourse.tile as tile
from concourse.vector_clock import ScopedClock
from concourse.bass_utils import run_bass_kernel_spmd

# ---- problem constants (hardcoded per contract) ----
N_CORES = 8
N_FULL = 32           # batch
CIN = 128             # input channels
COUT = 256            # output channels
H = W = 56
KH = KW = 3
BN_EPS = 1e-5

IMGS = N_FULL // N_CORES          # 4 images per core
WP = W + 2                        # 58 padded width
HP = H + 2
PADPIX = HP * WP                  # 3364
PIX = H * W                       # 3136
NCHUNK = COUT // 128              # 2 chunks of 128 output channels
RTR = 8                           # rows per matmul tile
RT = H // RTR                     # 7 row tiles per image
NTILE = RTR * W                   # 448 = matmul free dim (<=512, one PSUM bank)
NTOT = N_FULL * PIX               # 200704 elements per channel for BN stats
NPLANE = 4                        # lev1, lev2, lev3, lev3dup
NPAIR = 14                        # DoubleRow matmuls per PSUM tile

BF16 = mybir.dt.bfloat16
F32 = mybir.dt.float32
F8E4 = mybir.dt.float8e4
E4 = ml_dtypes.float8_e4m3

# pair schedule: (kind, a, b); kind "12" = (lev1,lev2) of tap a;
# kind "33" = (lev3 of tap a, lev3dup of tap b); kind "3z" = (lev3 of tap a, zeros)
PAIRS = ([("12", t, t) for t in range(9)]
         + [("33", 2 * i, 2 * i + 1) for i in range(4)]
         + [("3z", 8, 8)])

_MAX_DRAIN_WAITS = 1  # walrus CTRL instructions accept a single sync wait


def _split_multi_waits(nc, max_waits=1):
    """This walrus build rejects instructions with more than one sem wait.
    Hoist excess waits onto same-engine NoOps inserted immediately before the
    offending instruction (the engine blocks at the NoOp instead — identical
    ordering semantics)."""
    ctr = 0
    for bbw in nc.main_func.blocks:
        out = []
        changed = False
        for inst in bbw.instructions:
            si = inst.sync_info
            w = list(si.on_wait or []) if si else []
            if len(w) > max_waits:
                changed = True
                excess = w[: len(w) - max_waits]
                for i in range(0, len(excess), max_waits):
                    nop = mybir.InstNoOp(name=f"WFIX-{ctr}", ins=[], outs=[])
                    ctr += 1
                    nop.engine = inst.engine
                    nop.sync_info = mybir.SyncInfo(
                        on_wait=excess[i : i + max_waits], on_update=[]
                    )
                    out.append(nop)
                inst.sync_info = mybir.SyncInfo(
                    on_wait=w[len(w) - max_waits :],
                    on_update=list(si.on_update or []),
                )
            out.append(inst)
        if changed:
            bbw.instructions = out
    return ctr


class _SplitDrainTileContext(tile.TileContext):
    """TileContext whose final drain splits its sem waits across multiple
    sync-engine instructions (this walrus build caps CTRL waits at 1)."""

    def _drain_and_barrier(self, tick_clock, wait_clock):
        drain_inst = self.nc.sync.drain()
        wait_clock.add_sem_waits(
            drain_inst.ins, ScopedClock({None: tick_clock.global_clock})
        )
        si = drain_inst.ins.sync_info
        w = list(si.on_wait or [])
        if len(w) > _MAX_DRAIN_WAITS:
            drain_inst.ins.sync_info = mybir.SyncInfo(
                on_wait=w[:_MAX_DRAIN_WAITS], on_update=list(si.on_update or [])
            )
            # spread the remaining terminal-value waits across all engines so
            # they evaluate in parallel (each engine then syncs at the
            # barrier); a serial single-engine NoOp chain costs ~75ns/wait
            engines = [self.nc.sync, self.nc.vector, self.nc.scalar,
                       self.nc.gpsimd, self.nc.tensor]
            for k, i in enumerate(range(_MAX_DRAIN_WAITS, len(w),
                                        _MAX_DRAIN_WAITS)):
                eng = engines[k % len(engines)]
                nop = eng.nop(nofuse=True) if eng is self.nc.sync else eng.nop()
                nop.ins.sync_info = mybir.SyncInfo(
                    on_wait=w[i : i + _MAX_DRAIN_WAITS], on_update=[]
                )
        self.nc.all_engine_barrier()
        assert self.sems is not None
        popped = self.nc._tile_sem_poison_stack.pop()
        assert popped is self._sem_poison
        self.nc.clear_and_free_semaphores(list(self.sems.allocated().values()))
        self.nc.all_engine_barrier()


def _mv_pair_ap(xt_ap, pair, rt):
    """Moving AP [128, 2, 8, 56] for one DoubleRow matmul.

    xt_ap: AP of the per-image x tile [128, NPLANE*PADPIX] (planes:
    lev1, lev2, lev3, lev3dup). Row-tile rt covers padded rows
    rt*8+dy .. rt*8+dy+8, cols dx..dx+56 for tap (dy, dx)."""
    kind, ta, tb = pair
    dya, dxa = divmod(ta, KW)
    dyb, dxb = divmod(tb, KW)
    offa = (rt * RTR + dya) * WP + dxa
    offb = (rt * RTR + dyb) * WP + dxb
    if kind == "12":
        plane, stride = 0, PADPIX
    elif kind == "33":
        plane, stride = 2 * PADPIX, PADPIX + (offb - offa)
    else:  # "3z": second half reads lev3dup at same tap; weights are zero
        plane, stride = 2 * PADPIX, PADPIX
    dims = [list(xt_ap.ap[0]), [stride, 2], [WP, RTR], [1, W]]
    return bass_rust.AP(xt_ap.tensor, xt_ap.offset + plane + offa, dims)


def build_bass(n_cores=N_CORES, collective=True):
    """Build the per-core Bass module (SPMD: same program on every core)."""
    nc = bass.Bass(num_devices=n_cores)

    xq_d = nc.dram_tensor("xq", [IMGS, CIN, NPLANE * PADPIX], F8E4,
                          kind="ExternalInput")
    ws_d = nc.dram_tensor("ws", [CIN, NCHUNK * NPAIR * 2 * 128], F8E4,
                          kind="ExternalInput")
    abg_d = nc.dram_tensor("abg", [128, 3 * NCHUNK], F32, kind="ExternalInput")
    out_d = nc.dram_tensor("out", [IMGS, NCHUNK, 128, PIX], F8E4,
                           kind="ExternalOutput")

    with _SplitDrainTileContext(nc) as tc:
        with (
            tc.tile_pool(name="const", bufs=1) as constp,
            tc.tile_pool(name="xbuf", bufs=1) as xp,
            tc.tile_pool(name="zbuf", bufs=1) as zp,
            tc.tile_pool(name="stats", bufs=1) as sp,
            tc.tile_pool(name="sq", bufs=2) as sqp,
            tc.tile_pool(name="ostg", bufs=10) as op,
            tc.tile_pool(name="pz", bufs=8, space="PSUM") as pp,
            tc.tile_pool(name="dram", bufs=1, space="DRAM") as dp,
        ):
            # ---- constants + x tiles. DMA issue order is tuned so the
            # first conv tile's data (chunk-0 weights + img0 lev1/lev2
            # top rows) lands as early as possible: each HWDGE setup is
            # ~625ns serial, so first-needed transfers go first. ----
            w_sb = constp.tile([128, NCHUNK * NPAIR * 2 * 128], F8E4, tag="wsgn")
            abg_sb = constp.tile([128, 3 * NCHUNK], F32, tag="abg")
            xt = [xp.tile([128, NPLANE * PADPIX], F8E4, tag=f"x{img}",
                          name=f"x{img}") for img in range(IMGS)]

            WCH = NPAIR * 2 * 128          # weight cols per chunk

            def dma_x01_rows(img, r0, r1):
                """DMA planes 0-1 (lev1, lev2) rows r0..r1 of one image."""
                dst = xt[img][:, 0 : 2 * PADPIX].rearrange(
                    "p (pl hw) -> p pl hw", pl=2)[:, :, r0 * WP : r1 * WP]
                src = xq_d[img].rearrange(
                    "p (pl hw) -> p pl hw", pl=NPLANE)[:, 0:2, r0 * WP : r1 * WP]
                nc.sync.dma_start(dst, src)

            # PE p-state warmup: ~5us of tiny matmuls on a zeroed tile so the
            # Tensor engine reaches full clock before the first real matmul
            # (results accumulate into a never-read PSUM tile).
            warm = constp.tile([128, 192], F8E4, tag="warm")
            nc.vector.memset(warm[:], 0)
            wps = pp.tile([128, NTILE], F32, tag="pz", name="warmps")
            NWARM = 56
            for i in range(NWARM):
                nc.tensor.matmul(wps[:, 0:64], warm[:, 0:128], warm[:, 128:192],
                                 start=(i == 0), stop=(i == NWARM - 1))

            # any (lev1,lev2) pair matmul's dep-range spans all of lev1 plus
            # the head of lev2, so img0 loads as [lev1 | lev2 head | rest]
            nc.scalar.dma_start(w_sb[:, 0:512], ws_d[:, 0:512])    # pairs 0-1
            nc.sync.dma_start(xt[0][:, 0:PADPIX], xq_d[0, :, 0:PADPIX])
            nc.sync.dma_start(xt[0][:, PADPIX : PADPIX + 29 * WP],
                              xq_d[0, :, PADPIX : PADPIX + 29 * WP])
            nc.sync.dma_start(w_sb[:, 512:2048], ws_d[:, 512:2048])  # pairs 2-7
            nc.sync.dma_start(w_sb[:, 2048:WCH], ws_d[:, 2048:WCH])  # pairs 8-13
            nc.sync.dma_start(xt[0][:, PADPIX + 29 * WP : 2 * PADPIX],
                              xq_d[0, :, PADPIX + 29 * WP : 2 * PADPIX])
            nc.sync.dma_start(xt[0][:, 2 * PADPIX : 4 * PADPIX],
                              xq_d[0, :, 2 * PADPIX : 4 * PADPIX])
            for img in range(1, IMGS):
                nc.sync.dma_start(xt[img][:, 0 : 2 * PADPIX],
                                  xq_d[img, :, 0 : 2 * PADPIX])
                nc.sync.dma_start(xt[img][:, 2 * PADPIX : 4 * PADPIX],
                                  xq_d[img, :, 2 * PADPIX : 4 * PADPIX])
                if img == 1:
                    nc.sync.dma_start(w_sb[:, WCH : 2 * WCH],
                                      ws_d[:, WCH : 2 * WCH])      # chunk 1
            nc.sync.dma_start(abg_sb[:], abg_d[:])

            # ---- z buffers + stats ----
            z = [zp.tile([128, IMGS * PIX], F32, tag=f"z{j}", name=f"z{j}")
                 for j in range(NCHUNK)]
            ssum = sp.tile([128, 64], F32, tag="ssum")
            ssq = sp.tile([128, 64], F32, tag="ssq")

            al2 = abg_sb[:, 0:NCHUNK]                      # -alpha^2
            alga = abg_sb[:, NCHUNK : 2 * NCHUNK]          # alpha*gamma
            beta = abg_sb[:, 2 * NCHUNK : 3 * NCHUNK]
            inv_n = 1.0 / NTOT
            npart = IMGS * RT

            # Per chunk: conv -> stats AllReduce -> sign+store. Chunk 0's
            # collective + BN tail overlaps chunk 1's conv on PE.
            for j in range(NCHUNK):
                partial = sp.tile([128, 2], F32, tag=f"part{j}",
                                  name=f"part{j}")
                for img in range(IMGS):
                    for rt in range(RT):
                        if img == IMGS - 1 and rt == RT - 1:
                            # pre-reduce stat columns of the first 27 tiles
                            # while the PE runs the last tile; only a [128,1]
                            # add per stat remains on the critical path
                            nc.vector.reduce_sum(
                                out=partial[:, 0:1],
                                in_=ssum[:, j * npart : j * npart + npart - 1],
                                axis=mybir.AxisListType.X,
                            )
                            nc.vector.reduce_sum(
                                out=partial[:, 1:2],
                                in_=ssq[:, j * npart : j * npart + npart - 1],
                                axis=mybir.AxisListType.X,
                            )
                        pt = pp.tile([128, NTILE], F32, tag="pz",
                                     name=f"pz{j}_{img}_{rt}")
                        for pi, pair in enumerate(PAIRS):
                            wcol = ((j * NPAIR + pi) * 2) * 128
                            lhsT = w_sb[:, wcol : wcol + 256].rearrange(
                                "p (two m) -> p two m", two=2
                            )
                            rhs = _mv_pair_ap(xt[img][:], pair, rt)
                            nc.tensor.matmul(
                                pt[:], lhsT, rhs,
                                start=(pi == 0), stop=(pi == NPAIR - 1),
                                perf_mode=mybir.MatmulPerfMode.DoubleRow,
                            )
                        col = img * RT + rt
                        zs = z[j][:, img * PIX + rt * NTILE
                                  : img * PIX + (rt + 1) * NTILE]
                        nc.vector.tensor_scalar(
                            out=zs, in0=pt[:], scalar1=0.0, scalar2=None,
                            op0=mybir.AluOpType.add, op1=mybir.AluOpType.add,
                            accum_out=ssum[:, j * npart + col
                                           : j * npart + col + 1],
                        )
                        # square+accum alternates between ACT (reading the
                        # PSUM tile) and DVE (reading the SBUF z copy, after
                        # its own copy) to halve the ACT backlog; the last
                        # tile always goes to ACT so it runs in parallel
                        # with the final z-copy and stats close sooner
                        sq_acc = ssq[:, j * npart + col : j * npart + col + 1]
                        last = img == IMGS - 1 and rt == RT - 1
                        sqt = sqp.tile([128, NTILE], F32,
                                       tag="sqtl" if last else "sqt",
                                       name=f"sqt{j}_{img}_{rt}")
                        if col % 2 == 1 and not last:
                            nc.scalar.activation(
                                out=sqt[:], in_=pt[:],
                                func=mybir.ActivationFunctionType.Square,
                                accum_out=sq_acc,
                            )
                        else:
                            # last tile stays on DVE right behind its own
                            # z-copy: all waits already satisfied, no
                            # cross-engine sem on the stats critical path
                            nc.vector.scalar_tensor_tensor(
                                out=sqt[:], in0=zs, scalar=1.0, in1=zs,
                                op0=mybir.AluOpType.mult, op1=mybir.AluOpType.mult,
                                accum_out=sq_acc,
                            )

                # ---- chunk-j stats: [128,2] = (sum, sumsq) ----
                cc_sb = sp.tile([128, 2], F32, tag=f"ccsb{j}", name=f"ccsb{j}")
                lastc = j * npart + npart - 1
                nc.vector.tensor_tensor(
                    out=cc_sb[:, 0:1], in0=partial[:, 0:1],
                    in1=ssum[:, lastc : lastc + 1], op=mybir.AluOpType.add)
                nc.vector.tensor_tensor(
                    out=cc_sb[:, 1:2], in0=partial[:, 1:2],
                    in1=ssq[:, lastc : lastc + 1], op=mybir.AluOpType.add)
                if collective and n_cores > 1:
                    st = sp.tile([128, 2], F32, tag=f"st{j}", name=f"st{j}")
                    cc_in = dp.tile([128, 2], F32, tag=f"ccin{j}",
                                    name=f"ccin{j}")
                    cc_out = dp.tile([128, 2], F32, tag=f"ccout{j}",
                                     name=f"ccout{j}")
                    nc.sync.dma_start(cc_in[:], cc_sb[:])
                    nc.gpsimd.collective_compute(
                        "AllReduce", mybir.AluOpType.add,
                        replica_groups=[list(range(n_cores))],
                        ins=[cc_in.opt()], outs=[cc_out.opt()],
                    )
                    nc.sync.dma_start(st[:], cc_out[:])
                else:
                    st = cc_sb

                # ---- A, B for chunk j:  out = sign(z*A + B) ----
                # al2/alga precomputed on host; Rsqrt on ACT; rest on DVE.
                a2, ag, be = (v[:, j : j + 1] for v in (al2, alga, beta))
                mz = sp.tile([128, 2], F32, tag=f"mz{j}", name=f"mz{j}")
                var = sp.tile([128, 1], F32, tag=f"var{j}", name=f"var{j}")
                r = sp.tile([128, 1], F32, tag=f"r{j}", name=f"r{j}")
                A = sp.tile([128, 1], F32, tag=f"A{j}", name=f"A{j}")
                B = sp.tile([128, 1], F32, tag=f"B{j}", name=f"B{j}")
                T = sp.tile([128, 1], F32, tag=f"T{j}", name=f"T{j}")
                muA = sp.tile([128, 1], F32, tag=f"muA{j}", name=f"muA{j}")
                tmp = sp.tile([128, 1], F32, tag=f"tmp{j}", name=f"tmp{j}")

                nc.vector.tensor_scalar(out=mz[:], in0=st[:], scalar1=inv_n,
                                        scalar2=None, op0=mybir.AluOpType.mult)
                mu = mz[:, 0:1]
                # negvar = mu*mu - E[z^2]; d = negvar*(-al2) + eps
                # (a2 holds -alpha^2, precomputed on host)
                nc.vector.scalar_tensor_tensor(
                    out=var[:], in0=mu, scalar=mu, in1=mz[:, 1:2],
                    op0=mybir.AluOpType.mult, op1=mybir.AluOpType.subtract)
                nc.vector.tensor_scalar(out=var[:], in0=var[:], scalar1=a2,
                                        scalar2=float(BN_EPS),
                                        op0=mybir.AluOpType.mult,
                                        op1=mybir.AluOpType.add)
                nc.scalar.sqrt(r[:], var[:])
                nc.vector.reciprocal(r[:], r[:])
                nc.vector.tensor_tensor(out=A[:], in0=r[:], in1=ag,
                                        op=mybir.AluOpType.mult)
                nc.vector.tensor_tensor(out=muA[:], in0=mu, in1=A[:],
                                        op=mybir.AluOpType.mult)
                nc.vector.tensor_tensor(out=B[:], in0=be, in1=muA[:],
                                        op=mybir.AluOpType.subtract)
                # threshold T = -B/A = (mu*A - be)/A for the is_ge engines
                nc.vector.reciprocal(tmp[:], A[:])
                nc.vector.tensor_tensor(out=muA[:], in0=muA[:], in1=be,
                                        op=mybir.AluOpType.subtract)
                nc.vector.tensor_tensor(out=T[:], in0=muA[:], in1=tmp[:],
                                        op=mybir.AluOpType.mult)

                # ---- sign -> fp8 staging -> DRAM. Half-image slices spread
                # across ACT (Sign -> ±1), DVE and Pool (is_ge -> {0,1}; host
                # decodes 2v-1). Finer slices let the out-DMA pipeline drain
                # while later slices still compute. Pool's slice issues its
                # DMA through SWDGE, off the shared HWDGE device.
                HPIX = PIX // 2
                QPIX = PIX // 4
                # (sign engine, img, c0, c1, dma issuer) — DMA issue is
                # spread across engine DGE queues: scalar/vector use the
                # shared HWDGE device, gpsimd goes through SWDGE, so setups
                # overlap instead of serializing on one 625ns/DMA device.
                sign_jobs = [
                    ("act", 0, 0, HPIX, nc.scalar),
                    ("act", 0, HPIX, PIX, nc.scalar),
                    ("act", 1, HPIX, HPIX + QPIX, nc.scalar),
                    ("pool", 1, 0, HPIX, nc.gpsimd),
                    ("dve", 2, 0, HPIX, nc.sync),
                    ("dve", 2, HPIX, PIX, nc.sync),
                    ("dve", 3, 0, HPIX, nc.gpsimd),
                    ("dve", 3, HPIX, PIX, nc.sync),
                    ("dve", 1, HPIX + QPIX, PIX, nc.gpsimd),
                ]
                for eng, img, c0, c1, dma_eng in sign_jobs:
                    ostg = op.tile([128, c1 - c0], F8E4, tag=f"ostg{c1 - c0}",
                                   name=f"ostg{j}_{img}_{c0}")
                    zi = z[j][:, img * PIX + c0 : img * PIX + c1]
                    if eng == "act":
                        nc.scalar.activation(
                            out=ostg[:], in_=zi,
                            func=mybir.ActivationFunctionType.Sign,
                            bias=B[:, 0:1], scale=A[:, 0:1],
                        )
                    elif eng == "dve":
                        nc.vector.tensor_scalar(
                            out=ostg[:], in0=zi, scalar1=T[:, 0:1],
                            scalar2=None, op0=mybir.AluOpType.is_ge,
                        )
                    else:
                        nc.gpsimd.tensor_scalar(
                            out=ostg[:], in0=zi, scalar1=T[:, 0:1],
                            scalar2=None, op0=mybir.AluOpType.is_ge,
                        )
                    dma_eng.dma_start(out_d[img, j, :, c0:c1], ostg[:])

    _split_multi_waits(nc)
    return nc


def _prep_inputs(x, weight, gamma, beta):
    """Host-side prep: sign/alpha folding, padding, 3-level fp8 split."""
    x = np.ascontiguousarray(x, dtype=np.float32)
    weight = np.ascontiguousarray(weight, dtype=np.float32)

    alpha = np.abs(weight).mean(axis=(1, 2, 3)).astype(np.float32)      # [256]
    sgn = np.where(weight >= 0, np.float32(1), np.float32(-1))          # [256,128,3,3]
    sgn_t = sgn.transpose(1, 2, 3, 0).reshape(CIN, KH * KW, COUT)       # [cin,k,o]

    # weight pair blocks: [cin, chunk, pair, 2, 128] fp8
    ws = np.zeros((CIN, NCHUNK, NPAIR, 2, 128), np.float32)
    for j in range(NCHUNK):
        osl = slice(j * 128, (j + 1) * 128)
        for pi, (kind, ta, tb) in enumerate(PAIRS):
            if kind == "12":
                ws[:, j, pi, 0] = sgn_t[:, ta, osl]
                ws[:, j, pi, 1] = sgn_t[:, ta, osl] * 2.0**-4
            elif kind == "33":
                ws[:, j, pi, 0] = sgn_t[:, ta, osl] * 2.0**-8
                ws[:, j, pi, 1] = sgn_t[:, tb, osl] * 2.0**-8
            else:  # "3z"
                ws[:, j, pi, 0] = sgn_t[:, ta, osl] * 2.0**-8
    ws = np.ascontiguousarray(ws.reshape(CIN, NCHUNK * NPAIR * 2 * 128)).astype(E4)

    # abg[p, j] layout: [alpha^2(2) | alpha*gamma(2) | beta(2)], o = j*128+p
    def chunked(v):
        return np.ascontiguousarray(v.reshape(NCHUNK, 128).T)  # [128, 2]
    abg = np.concatenate(
        [chunked(-alpha * alpha),
         chunked(alpha * np.asarray(gamma, np.float32)),
         chunked(np.asarray(beta, np.float32))], axis=1
    ).astype(np.float32)                                                # [128, 6]

    # 3-level e4m3 split of padded x (+ duplicated lev3 plane)
    xpad = np.zeros((N_FULL, CIN, HP, WP), np.float32)
    xpad[:, :, 1 : H + 1, 1 : W + 1] = x
    x1 = xpad.astype(E4)
    r1 = xpad - x1.astype(np.float32)
    x2 = (r1 * 2.0**4).astype(E4)
    r2 = r1 - x2.astype(np.float32) * 2.0**-4
    x3 = (r2 * 2.0**8).astype(E4)
    xq = np.stack([x1, x2, x3, x3], axis=2)          # [N, CIN, 4, HP, WP]
    xq = xq.reshape(N_FULL, CIN, NPLANE * PADPIX)

    in_maps = []
    for c in range(N_CORES):
        sl = slice(c * IMGS, (c + 1) * IMGS)
        in_maps.append({
            "xq": np.ascontiguousarray(xq[sl]),
            "ws": ws,
            "abg": abg,
        })
    return in_maps


def kernel(x, weight, gamma, beta):
    in_maps = _prep_inputs(x, weight, gamma, beta)
    nc = build_bass()
    res = run_bass_kernel_spmd(nc, in_maps, core_ids=list(range(N_CORES)))
    out = np.empty((N_FULL, COUT, H, W), np.float32)
    hp, qp = PIX // 2, PIX // 4
    for c in range(N_CORES):
        o = res.results[c]["out"].astype(np.float32)   # [IMGS, 2, 128, 3136]
        # ACT slices emit ±1 (img0 and img1 cols [hp, hp+qp)); all other
        # slices are is_ge {0,1} and decode as 2v-1
        act_quarter = o[1, :, :, hp : hp + qp].copy()
        o[1:4] = 2.0 * o[1:4] - 1.0
        o[1, :, :, hp : hp + qp] = act_quarter
        o = o.reshape(IMGS, COUT, H, W)
        out[c * IMGS : (c + 1) * IMGS] = o
    return out


# revision 72
# speedup vs baseline: 1.0013x; 1.0013x over previous
"""Trainium2 Bass kernel for ConvBnSign (binarized 3x3 conv + sync-BN + sign).

Math: y = conv2d(x, sign(w) * alpha)  with alpha = mean|w| per out-channel,
then train-mode BatchNorm over (N,H,W), then hard_sign.

Since alpha_o > 0 is a per-channel scale, fold it into the BN affine:
  z = conv2d(x, sign(w))
  out = sign(z*A + B),  A = alpha*gamma*rsqrt(alpha^2 var_z + eps),
                        B = beta - mu_z*A

Precision: x is split on host into 3 fp8-e4m3 planes (lev1 = e4m3(x),
lev2 = e4m3(r1*2^4), lev3 = e4m3(r2*2^8)); reconstruction error rms
~2^-16 relative, which flips only ~4e-6 of the output signs.

Conv engine schedule: every 128-contraction k-tile (9 taps x 3 levels = 27
per output tile, plus one zero-padded slot) is executed via fp8 DoubleRow
matmuls that process TWO k-tiles per instruction at 0.5 cycles/row:
  - 9 pairs  (lev1, lev2) of the same tap, pair-stride = PADPIX
  - 4 pairs  (lev3 tap 2i, lev3' tap 2i+1) using a duplicated lev3 plane
    so the pair stride stays collision-free
  - 1 pair   (lev3 tap 8, zero weights)
=> 14 matmuls per [128,448] PSUM tile instead of 18 bf16 matmuls.

Sharding: data-parallel, 4 images per core across 8 cores; BN stats are
per-channel partial sums [128,2] fp32 all-reduced across cores.

Other schedule details:
  - ~3us of tiny warmup matmuls on a zeroed tile bring the PE out of its
    low p-states while the first input DMAs are still in flight
  - input DMA order is tuned around the dep-tracker's contiguous-range
    granularity (img0 lev1 whole, then lev2 head rows, then the rest)
  - BN stats: z-copy+sum-accum on DVE from PSUM; square+sumsq-accum
    alternates ACT (from PSUM) / DVE (from the z copy; GPSIMD cannot
    access PSUM), with the last tile on DVE right behind its own copy
  - sign outputs are fp8 half/quarter-image slices spread over ACT
    (Sign -> ±1), DVE and Pool (is_ge(T) -> {0,1}, host decodes 2v-1),
    with out-DMAs issued through both HWDGE and Pool's SWDGE queues
"""

import numpy as np
import ml_dtypes
import bass_rust

import concourse.bass as bass
import concourse.mybir as mybir
import concourse.tile as tile
from concourse.vector_clock import ScopedClock
from concourse.bass_utils import run_bass_kernel_spmd

# ---- problem constants (hardcoded per contract) ----
N_CORES = 8
N_FULL = 32           # batch
CIN = 128             # input channels
COUT = 256            # output channels
H = W = 56
KH = KW = 3
BN_EPS = 1e-5

IMGS = N_FULL // N_CORES          # 4 images per core
WP = W + 2                        # 58 padded width
HP = H + 2
PADPIX = HP * WP                  # 3364
PIX = H * W                       # 3136
NCHUNK = COUT // 128              # 2 chunks of 128 output channels
RTR = 8                           # rows per matmul tile
RT = H // RTR                     # 7 row tiles per image
NTILE = RTR * W                   # 448 = matmul free dim (<=512, one PSUM bank)
NTOT = N_FULL * PIX               # 200704 elements per channel for BN stats
NPLANE = 4                        # lev1, lev2, lev3, lev3dup
NPAIR = 14                        # DoubleRow matmuls per PSUM tile

BF16 = mybir.dt.bfloat16
F32 = mybir.dt.float32
F8E4 = mybir.dt.float8e4
E4 = ml_dtypes.float8_e4m3

# pair schedule: (kind, a, b); kind "12" = (lev1,lev2) of tap a;
# kind "33" = (lev3 of tap a, lev3dup of tap b); kind "3z" = (lev3 of tap a, zeros)
PAIRS = ([("12", t, t) for t in range(9)]
         + [("33", 2 * i, 2 * i + 1) for i in range(4)]
         + [("3z", 8, 8)])

_MAX_DRAIN_WAITS = 1  # walrus CTRL instructions accept a single sync wait


def _split_multi_waits(nc, max_waits=1):
    """This walrus build rejects instructions with more than one sem wait.
    Hoist excess waits onto same-engine NoOps inserted immediately before the
    offending instruction (the engine blocks at the NoOp instead — identical
    ordering semantics)."""
    ctr = 0
    for bbw in nc.main_func.blocks:
        out = []
        changed = False
        for inst in bbw.instructions:
            si = inst.sync_info
            w = list(si.on_wait or []) if si else []
            if len(w) > max_waits:
                changed = True
                excess = w[: len(w) - max_waits]
                for i in range(0, len(excess), max_waits):
                    nop = mybir.InstNoOp(name=f"WFIX-{ctr}", ins=[], outs=[])
                    ctr += 1
                    nop.engine = inst.engine
                    nop.sync_info = mybir.SyncInfo(
                        on_wait=excess[i : i + max_waits], on_update=[]
                    )
                    out.append(nop)
                inst.sync_info = mybir.SyncInfo(
                    on_wait=w[len(w) - max_waits :],
                    on_update=list(si.on_update or []),
                )
            out.append(inst)
        if changed:
            bbw.instructions = out
    return ctr


class _SplitDrainTileContext(tile.TileContext):
    """TileContext whose final drain splits its sem waits across multiple
    sync-engine instructions (this walrus build caps CTRL waits at 1)."""

    def _drain_and_barrier(self, tick_clock, wait_clock):
        drain_inst = self.nc.sync.drain()
        wait_clock.add_sem_waits(
            drain_inst.ins, ScopedClock({None: tick_clock.global_clock})
        )
        si = drain_inst.ins.sync_info
        w = list(si.on_wait or [])
        if len(w) > _MAX_DRAIN_WAITS:
            drain_inst.ins.sync_info = mybir.SyncInfo(
                on_wait=w[:_MAX_DRAIN_WAITS], on_update=list(si.on_update or [])
            )
            # spread the remaining terminal-value waits across all engines so
            # they evaluate in parallel (each engine then syncs at the
            # barrier); a serial single-engine NoOp chain costs ~75ns/wait
            engines = [self.nc.sync, self.nc.vector, self.nc.scalar,
                       self.nc.gpsimd, self.nc.tensor]
            for k, i in enumerate(range(_MAX_DRAIN_WAITS, len(w),
                                        _MAX_DRAIN_WAITS)):
                eng = engines[k % len(engines)]
                nop = eng.nop(nofuse=True) if eng is self.nc.sync else eng.nop()
                nop.ins.sync_info = mybir.SyncInfo(
                    on_wait=w[i : i + _MAX_DRAIN_WAITS], on_update=[]
                )
        self.nc.all_engine_barrier()
        assert self.sems is not None
        popped = self.nc._tile_sem_poison_stack.pop()
        assert popped is self._sem_poison
        self.nc.clear_and_free_semaphores(list(self.sems.allocated().values()))
        self.nc.all_engine_barrier()


def _mv_pair_ap(xt_ap, pair, rt):
    """Moving AP [128, 2, 8, 56] for one DoubleRow matmul.

    xt_ap: AP of the per-image x tile [128, NPLANE*PADPIX] (planes:
    lev1, lev2, lev3, lev3dup). Row-tile rt covers padded rows
    rt*8+dy .. rt*8+dy+8, cols dx..dx+56 for tap (dy, dx)."""
    kind, ta, tb = pair
    dya, dxa = divmod(ta, KW)
    dyb, dxb = divmod(tb, KW)
    offa = (rt * RTR + dya) * WP + dxa
    offb = (rt * RTR + dyb) * WP + dxb
    if kind == "12":
        plane, stride = 0, PADPIX
    elif kind == "33":
        plane, stride = 2 * PADPIX, PADPIX + (offb - offa)
    else:  # "3z": second half reads lev3dup at same tap; weights are zero
        plane, stride = 2 * PADPIX, PADPIX
    dims = [list(xt_ap.ap[0]), [stride, 2], [WP, RTR], [1, W]]
    return bass_rust.AP(xt_ap.tensor, xt_ap.offset + plane + offa, dims)


def build_bass(n_cores=N_CORES, collective=True):
    """Build the per-core Bass module (SPMD: same program on every core)."""
    nc = bass.Bass(num_devices=n_cores)

    xq_d = nc.dram_tensor("xq", [IMGS, CIN, NPLANE * PADPIX], F8E4,
                          kind="ExternalInput")
    ws_d = nc.dram_tensor("ws", [CIN, NCHUNK * NPAIR * 2 * 128], F8E4,
                          kind="ExternalInput")
    abg_d = nc.dram_tensor("abg", [128, 3 * NCHUNK], F32, kind="ExternalInput")
    out_d = nc.dram_tensor("out", [IMGS, NCHUNK, 128, PIX], F8E4,
                           kind="ExternalOutput")

    with _SplitDrainTileContext(nc) as tc:
        with (
            tc.tile_pool(name="const", bufs=1) as constp,
            tc.tile_pool(name="xbuf", bufs=1) as xp,
            tc.tile_pool(name="zbuf", bufs=1) as zp,
            tc.tile_pool(name="stats", bufs=1) as sp,
            tc.tile_pool(name="sq", bufs=2) as sqp,
            tc.tile_pool(name="ostg", bufs=12) as op,
            tc.tile_pool(name="pz", bufs=8, space="PSUM") as pp,
            tc.tile_pool(name="dram", bufs=1, space="DRAM") as dp,
        ):
            # ---- constants + x tiles. DMA issue order is tuned so the
            # first conv tile's data (chunk-0 weights + img0 lev1/lev2
            # top rows) lands as early as possible: each HWDGE setup is
            # ~625ns serial, so first-needed transfers go first. ----
            w_sb = constp.tile([128, NCHUNK * NPAIR * 2 * 128], F8E4, tag="wsgn")
            abg_sb = constp.tile([128, 3 * NCHUNK], F32, tag="abg")
            xt = [xp.tile([128, NPLANE * PADPIX], F8E4, tag=f"x{img}",
                          name=f"x{img}") for img in range(IMGS)]

            WCH = NPAIR * 2 * 128          # weight cols per chunk

            def dma_x01_rows(img, r0, r1):
                """DMA planes 0-1 (lev1, lev2) rows r0..r1 of one image."""
                dst = xt[img][:, 0 : 2 * PADPIX].rearrange(
                    "p (pl hw) -> p pl hw", pl=2)[:, :, r0 * WP : r1 * WP]
                src = xq_d[img].rearrange(
                    "p (pl hw) -> p pl hw", pl=NPLANE)[:, 0:2, r0 * WP : r1 * WP]
                nc.sync.dma_start(dst, src)

            # PE p-state warmup: ~5us of tiny matmuls on a zeroed tile so the
            # Tensor engine reaches full clock before the first real matmul
            # (results accumulate into a never-read PSUM tile).
            warm = constp.tile([128, 192], F8E4, tag="warm")
            nc.vector.memset(warm[:], 0)
            wps = pp.tile([128, NTILE], F32, tag="pz", name="warmps")
            NWARM = 56
            for i in range(NWARM):
                nc.tensor.matmul(wps[:, 0:64], warm[:, 0:128], warm[:, 128:192],
                                 start=(i == 0), stop=(i == NWARM - 1))

            # any (lev1,lev2) pair matmul's dep-range spans all of lev1 plus
            # the head of lev2, so img0 loads as [lev1 | lev2 head | rest]
            nc.scalar.dma_start(w_sb[:, 0:512], ws_d[:, 0:512])    # pairs 0-1
            nc.sync.dma_start(xt[0][:, 0:PADPIX], xq_d[0, :, 0:PADPIX])
            nc.sync.dma_start(xt[0][:, PADPIX : PADPIX + 29 * WP],
                              xq_d[0, :, PADPIX : PADPIX + 29 * WP])
            nc.sync.dma_start(w_sb[:, 512:2048], ws_d[:, 512:2048])  # pairs 2-7
            nc.sync.dma_start(w_sb[:, 2048:WCH], ws_d[:, 2048:WCH])  # pairs 8-13
            nc.sync.dma_start(xt[0][:, PADPIX + 29 * WP : 2 * PADPIX],
                              xq_d[0, :, PADPIX + 29 * WP : 2 * PADPIX])
            nc.sync.dma_start(xt[0][:, 2 * PADPIX : 4 * PADPIX],
                              xq_d[0, :, 2 * PADPIX : 4 * PADPIX])
            for img in range(1, IMGS):
                nc.sync.dma_start(xt[img][:, 0 : 2 * PADPIX],
                                  xq_d[img, :, 0 : 2 * PADPIX])
                nc.sync.dma_start(xt[img][:, 2 * PADPIX : 4 * PADPIX],
                                  xq_d[img, :, 2 * PADPIX : 4 * PADPIX])
                if img == 1:
                    nc.sync.dma_start(w_sb[:, WCH : 2 * WCH],
                                      ws_d[:, WCH : 2 * WCH])      # chunk 1
            nc.sync.dma_start(abg_sb[:], abg_d[:])

            # ---- z buffers + stats ----
            z = [zp.tile([128, IMGS * PIX], F32, tag=f"z{j}", name=f"z{j}")
                 for j in range(NCHUNK)]
            ssum = sp.tile([128, 64], F32, tag="ssum")
            ssq = sp.tile([128, 64], F32, tag="ssq")

            al2 = abg_sb[:, 0:NCHUNK]                      # -alpha^2
            alga = abg_sb[:, NCHUNK : 2 * NCHUNK]          # alpha*gamma
            beta = abg_sb[:, 2 * NCHUNK : 3 * NCHUNK]
            inv_n = 1.0 / NTOT
            npart = IMGS * RT

            # Per chunk: conv -> stats AllReduce -> sign+store. Chunk 0's
            # collective + BN tail overlaps chunk 1's conv on PE.
            for j in range(NCHUNK):
                partial = sp.tile([128, 2], F32, tag=f"part{j}",
                                  name=f"part{j}")
                for img in range(IMGS):
                    for rt in range(RT):
                        if img == IMGS - 1 and rt == RT - 1:
                            # pre-reduce stat columns of the first 27 tiles
                            # while the PE runs the last tile; only a [128,1]
                            # add per stat remains on the critical path
                            nc.vector.reduce_sum(
                                out=partial[:, 0:1],
                                in_=ssum[:, j * npart : j * npart + npart - 1],
                                axis=mybir.AxisListType.X,
                            )
                            nc.vector.reduce_sum(
                                out=partial[:, 1:2],
                                in_=ssq[:, j * npart : j * npart + npart - 1],
                                axis=mybir.AxisListType.X,
                            )
                        pt = pp.tile([128, NTILE], F32, tag="pz",
                                     name=f"pz{j}_{img}_{rt}")
                        for pi, pair in enumerate(PAIRS):
                            wcol = ((j * NPAIR + pi) * 2) * 128
                            lhsT = w_sb[:, wcol : wcol + 256].rearrange(
                                "p (two m) -> p two m", two=2
                            )
                            rhs = _mv_pair_ap(xt[img][:], pair, rt)
                            nc.tensor.matmul(
                                pt[:], lhsT, rhs,
                                start=(pi == 0), stop=(pi == NPAIR - 1),
                                perf_mode=mybir.MatmulPerfMode.DoubleRow,
                            )
                        col = img * RT + rt
                        zs = z[j][:, img * PIX + rt * NTILE
                                  : img * PIX + (rt + 1) * NTILE]
                        nc.vector.tensor_scalar(
                            out=zs, in0=pt[:], scalar1=0.0, scalar2=None,
                            op0=mybir.AluOpType.add, op1=mybir.AluOpType.add,
                            accum_out=ssum[:, j * npart + col
                                           : j * npart + col + 1],
                        )
                        # square+accum alternates between ACT (reading the
                        # PSUM tile) and DVE (reading the SBUF z copy, after
                        # its own copy) to halve the ACT backlog; the last
                        # tile always goes to ACT so it runs in parallel
                        # with the final z-copy and stats close sooner
                        sq_acc = ssq[:, j * npart + col : j * npart + col + 1]
                        last = img == IMGS - 1 and rt == RT - 1
                        sqt = sqp.tile([128, NTILE], F32,
                                       tag="sqtl" if last else "sqt",
                                       name=f"sqt{j}_{img}_{rt}")
                        if not last:
                            nc.scalar.activation(
                                out=sqt[:], in_=pt[:],
                                func=mybir.ActivationFunctionType.Square,
                                accum_out=sq_acc,
                            )
                        else:
                            # last tile stays on DVE right behind its own
                            # z-copy: all waits already satisfied, no
                            # cross-engine sem on the stats critical path
                            nc.vector.scalar_tensor_tensor(
                                out=sqt[:], in0=zs, scalar=1.0, in1=zs,
                                op0=mybir.AluOpType.mult, op1=mybir.AluOpType.mult,
                                accum_out=sq_acc,
                            )

                # ---- chunk-j stats: [128,2] = (sum, sumsq) ----
                cc_sb = sp.tile([128, 2], F32, tag=f"ccsb{j}", name=f"ccsb{j}")
                lastc = j * npart + npart - 1
                nc.vector.tensor_tensor(
                    out=cc_sb[:, 0:1], in0=partial[:, 0:1],
                    in1=ssum[:, lastc : lastc + 1], op=mybir.AluOpType.add)
                nc.vector.tensor_tensor(
                    out=cc_sb[:, 1:2], in0=partial[:, 1:2],
                    in1=ssq[:, lastc : lastc + 1], op=mybir.AluOpType.add)
                if collective and n_cores > 1:
                    st = sp.tile([128, 2], F32, tag=f"st{j}", name=f"st{j}")
                    cc_in = dp.tile([128, 2], F32, tag=f"ccin{j}",
                                    name=f"ccin{j}")
                    cc_out = dp.tile([128, 2], F32, tag=f"ccout{j}",
                                     name=f"ccout{j}")
                    nc.sync.dma_start(cc_in[:], cc_sb[:])
                    nc.gpsimd.collective_compute(
                        "AllReduce", mybir.AluOpType.add,
                        replica_groups=[list(range(n_cores))],
                        ins=[cc_in.opt()], outs=[cc_out.opt()],
                    )
                    nc.sync.dma_start(st[:], cc_out[:])
                else:
                    st = cc_sb

                # ---- A, B for chunk j:  out = sign(z*A + B) ----
                # al2/alga precomputed on host; Rsqrt on ACT; rest on DVE.
                a2, ag, be = (v[:, j : j + 1] for v in (al2, alga, beta))
                mz = sp.tile([128, 2], F32, tag=f"mz{j}", name=f"mz{j}")
                var = sp.tile([128, 1], F32, tag=f"var{j}", name=f"var{j}")
                r = sp.tile([128, 1], F32, tag=f"r{j}", name=f"r{j}")
                A = sp.tile([128, 1], F32, tag=f"A{j}", name=f"A{j}")
                B = sp.tile([128, 1], F32, tag=f"B{j}", name=f"B{j}")
                T = sp.tile([128, 1], F32, tag=f"T{j}", name=f"T{j}")
                muA = sp.tile([128, 1], F32, tag=f"muA{j}", name=f"muA{j}")
                tmp = sp.tile([128, 1], F32, tag=f"tmp{j}", name=f"tmp{j}")

                nc.vector.tensor_scalar(out=mz[:], in0=st[:], scalar1=inv_n,
                                        scalar2=None, op0=mybir.AluOpType.mult)
                mu = mz[:, 0:1]
                # negvar = mu*mu - E[z^2]; d = negvar*(-al2) + eps
                # (a2 holds -alpha^2, precomputed on host)
                nc.vector.scalar_tensor_tensor(
                    out=var[:], in0=mu, scalar=mu, in1=mz[:, 1:2],
                    op0=mybir.AluOpType.mult, op1=mybir.AluOpType.subtract)
                nc.vector.tensor_scalar(out=var[:], in0=var[:], scalar1=a2,
                                        scalar2=float(BN_EPS),
                                        op0=mybir.AluOpType.mult,
                                        op1=mybir.AluOpType.add)
                nc.scalar.sqrt(r[:], var[:])
                nc.vector.reciprocal(r[:], r[:])
                nc.vector.tensor_tensor(out=A[:], in0=r[:], in1=ag,
                                        op=mybir.AluOpType.mult)
                nc.vector.tensor_tensor(out=muA[:], in0=mu, in1=A[:],
                                        op=mybir.AluOpType.mult)
                nc.vector.tensor_tensor(out=B[:], in0=be, in1=muA[:],
                                        op=mybir.AluOpType.subtract)
                # threshold T = -B/A = (mu*A - be)/A for the is_ge engines
                nc.vector.reciprocal(tmp[:], A[:])
                nc.vector.tensor_tensor(out=muA[:], in0=muA[:], in1=be,
                                        op=mybir.AluOpType.subtract)
                nc.vector.tensor_tensor(out=T[:], in0=muA[:], in1=tmp[:],
                                        op=mybir.AluOpType.mult)

                # ---- sign -> fp8 staging -> DRAM. Half-image slices spread
                # across ACT (Sign -> ±1), DVE and Pool (is_ge -> {0,1}; host
                # decodes 2v-1). Finer slices let the out-DMA pipeline drain
                # while later slices still compute. Pool's slice issues its
                # DMA through SWDGE, off the shared HWDGE device.
                HPIX = PIX // 2
                QPIX = PIX // 4
                # (sign engine, img, c0, c1, dma issuer) — DMA issue is
                # spread across engine DGE queues: scalar/vector use the
                # shared HWDGE device, gpsimd goes through SWDGE, so setups
                # overlap instead of serializing on one 625ns/DMA device.
                sign_jobs = [
                    ("act", 0, 0, HPIX, nc.scalar),
                    ("act", 0, HPIX, PIX, nc.scalar),
                    ("act", 1, HPIX, HPIX + QPIX, nc.scalar),
                    ("pool", 1, 0, HPIX, nc.gpsimd),
                    ("dve", 2, 0, HPIX, nc.sync),
                    ("dve", 2, HPIX, PIX, nc.sync),
                    ("dve", 3, 0, HPIX, nc.gpsimd),
                    ("dve", 3, HPIX, PIX, nc.sync),
                    ("dve", 1, HPIX + QPIX, PIX, nc.gpsimd),
                ]
                for eng, img, c0, c1, dma_eng in sign_jobs:
                    ostg = op.tile([128, c1 - c0], F8E4, tag=f"ostg{c1 - c0}",
                                   name=f"ostg{j}_{img}_{c0}")
                    zi = z[j][:, img * PIX + c0 : img * PIX + c1]
                    if eng == "act":
                        nc.scalar.activation(
                            out=ostg[:], in_=zi,
                            func=mybir.ActivationFunctionType.Sign,
                            bias=B[:, 0:1], scale=A[:, 0:1],
                        )
                    elif eng == "dve":
                        nc.vector.tensor_scalar(
                            out=ostg[:], in0=zi, scalar1=T[:, 0:1],
                            scalar2=None, op0=mybir.AluOpType.is_ge,
                        )
                    else:
                        nc.gpsimd.tensor_scalar(
                            out=ostg[:], in0=zi, scalar1=T[:, 0:1],
                            scalar2=None, op0=mybir.AluOpType.is_ge,
                        )
                    dma_eng.dma_start(out_d[img, j, :, c0:c1], ostg[:])

    _split_multi_waits(nc)
    return nc


def _prep_inputs(x, weight, gamma, beta):
    """Host-side prep: sign/alpha folding, padding, 3-level fp8 split."""
    x = np.ascontiguousarray(x, dtype=np.float32)
    weight = np.ascontiguousarray(weight, dtype=np.float32)

    alpha = np.abs(weight).mean(axis=(1, 2, 3)).astype(np.float32)      # [256]
    sgn = np.where(weight >= 0, np.float32(1), np.float32(-1))          # [256,128,3,3]
    sgn_t = sgn.transpose(1, 2, 3, 0).reshape(CIN, KH * KW, COUT)       # [cin,k,o]

    # weight pair blocks: [cin, chunk, pair, 2, 128] fp8
    ws = np.zeros((CIN, NCHUNK, NPAIR, 2, 128), np.float32)
    for j in range(NCHUNK):
        osl = slice(j * 128, (j + 1) * 128)
        for pi, (kind, ta, tb) in enumerate(PAIRS):
            if kind == "12":
                ws[:, j, pi, 0] = sgn_t[:, ta, osl]
                ws[:, j, pi, 1] = sgn_t[:, ta, osl] * 2.0**-4
            elif kind == "33":
                ws[:, j, pi, 0] = sgn_t[:, ta, osl] * 2.0**-8
                ws[:, j, pi, 1] = sgn_t[:, tb, osl] * 2.0**-8
            else:  # "3z"
                ws[:, j, pi, 0] = sgn_t[:, ta, osl] * 2.0**-8
    ws = np.ascontiguousarray(ws.reshape(CIN, NCHUNK * NPAIR * 2 * 128)).astype(E4)

    # abg[p, j] layout: [alpha^2(2) | alpha*gamma(2) | beta(2)], o = j*128+p
    def chunked(v):
        return np.ascontiguousarray(v.reshape(NCHUNK, 128).T)  # [128, 2]
    abg = np.concatenate(
        [chunked(-alpha * alpha),
         chunked(alpha * np.asarray(gamma, np.float32)),
         chunked(np.asarray(beta, np.float32))], axis=1
    ).astype(np.float32)                                                # [128, 6]

    # 3-level e4m3 split of padded x (+ duplicated lev3 plane)
    xpad = np.zeros((N_FULL, CIN, HP, WP), np.float32)
    xpad[:, :, 1 : H + 1, 1 : W + 1] = x
    x1 = xpad.astype(E4)
    r1 = xpad - x1.astype(np.float32)
    x2 = (r1 * 2.0**4).astype(E4)
    r2 = r1 - x2.astype(np.float32) * 2.0**-4
    x3 = (r2 * 2.0**8).astype(E4)
    xq = np.stack([x1, x2, x3, x3], axis=2)          # [N, CIN, 4, HP, WP]
    xq = xq.reshape(N_FULL, CIN, NPLANE * PADPIX)

    in_maps = []
    for c in range(N_CORES):
        sl = slice(c * IMGS, (c + 1) * IMGS)
        in_maps.append({
            "xq": np.ascontiguousarray(xq[sl]),
            "ws": ws,
            "abg": abg,
        })
    return in_maps


def kernel(x, weight, gamma, beta):
    in_maps = _prep_inputs(x, weight, gamma, beta)
    nc = build_bass()
    res = run_bass_kernel_spmd(nc, in_maps, core_ids=list(range(N_CORES)))
    out = np.empty((N_FULL, COUT, H, W), np.float32)
    hp, qp = PIX // 2, PIX // 4
    for c in range(N_CORES):
        o = res.results[c]["out"].astype(np.float32)   # [IMGS, 2, 128, 3136]
        # ACT slices emit ±1 (img0 and img1 cols [hp, hp+qp)); all other
        # slices are is_ge {0,1} and decode as 2v-1
        act_quarter = o[1, :, :, hp : hp + qp].copy()
        o[1:4] = 2.0 * o[1:4] - 1.0
        o[1, :, :, hp : hp + qp] = act_quarter
        o = o.reshape(IMGS, COUT, H, W)
        out[c * IMGS : (c + 1) * IMGS] = o
    return out
